# revision 53
# baseline (speedup 1.0000x reference)
"""DualAttention Trainium2 kernel: 8-core data-parallel over batch.

Each NeuronCore processes one batch element [1024, 512]. Host-side
marshalling transposes the three input tensors to [d, token] layout and
quantizes to bf16 (fp8 where the error budget allows: the spatial-gate
MLP runs fp8 DoubleRow end to end, with Wv@Ws1 folded on the host so
its first matmul contracts the raw fp8 value in two 256-deep passes).

Input DMAs are spread across the SP/ACT/Pool queues (a DMA occupies its
issuing queue for the whole transfer), value-path tensors first, so all
inputs land ~4x sooner than a single-queue cascade. ACT's queue opens
with a dummy sigmoid and swaps to the exp table set via a dummy exp
pinned (by a data dependency) behind the last sigmoid: two table loads
total, both off the critical path.

All activations stay "transposed" (d on partitions, tokens free) so
every projection contracts over the partition dim; attention scores are
computed transposed ([k_tok, q_tok]). PV uses the probability chunk as
the stationary operand and v as the moving operand, producing [q_tok,
d] blocks: 64-column streams fill all 128 output partitions, halving
PV's PE time vs the [d, q] orientation. Softmax denominators accumulate
in a dedicated PSUM bank via ap=1 ones-column matmuls that reuse the
loaded stationary (~free). PSUM start=True clears the WHOLE bank's
has_written bits, so only the first write of each accumulation pass
carries it. Normalization is a per-partition reciprocal, one bf16 copy
of the numerators to SBUF, and GpSimd scalar-multiplies (GpSimd cannot
touch PSUM); head pairs are then PE-transposed back to [d, token] for
the output projection.

The attention stream is a flat software pipeline: per position, PV
(trailing PV_LAG chunks), one scores chunk, and paced filler work --
half-projections of the q/k weights through the single-bank psT pool
and k<=2 output-projection partials -- sized so ~2 positions of PE work
cover the exp turnaround of the 2-deep scores ring. Exps run on ACT
except three Schraudolph chunks per head on DVE (bf16 bits via int16
truncation). The tail folds the k=0..2 partials into each per-q8 k3
group with an identity matmul on the idle PE, so the final moves are
plain copies alternating ACT/DVE, each followed by its own DMA on the
opposite queue.
"""
import numpy as np
import ml_dtypes

import concourse.bass as bass
import concourse.tile as tile
from concourse import bacc, mybir
from concourse.bass_utils import run_bass_kernel_spmd
from concourse.masks import make_identity

B, N, D = 8, 1024, 512
H, DH, HID = 8, 64, 256
P = 128
K4 = D // P          # 4 d-chunks
T8 = N // P          # 8 token chunks
M2 = HID // P        # 2 hidden chunks
NCORES = 8
PV_LAG = 6
WARMUP = 0          # dummy transposes to ramp the PE p-state

F32 = mybir.dt.float32
F8 = mybir.dt.float8e4
BF16 = mybir.dt.bfloat16
I16 = mybir.dt.int16
AF = mybir.ActivationFunctionType
OP = mybir.AluOpType
AX = mybir.AxisListType
PM = mybir.MatmulPerfMode

F8NP = ml_dtypes.float8_e4m3
BF16NP = ml_dtypes.bfloat16

# Schraudolph exp in bf16 bits: i16 = trunc(x*0.125*log2e*128 + K2)
LOG2E = 1.4426950408889634
SCH_K1 = 0.125 * LOG2E * 128.0
SCH_K2 = 127.0 * 128.0 - 7.0     # -7 centers the multiplicative bias

# k8 positions per head whose exp runs as Schraudolph on DVE (GpSimd
# cannot touch PSUM, so the scores psum is only reachable from ACT/DVE)
DVE_K8 = (1, 4, 6)

WEIGHT_NAMES = ["Wq", "Wk", "Wv", "Wo", "Ws1", "Ws2", "Wc1", "Wc2"]
BIAS_NAMES = ["bq", "bk", "bv", "bo", "bs1", "bs2", "bc1", "bc2"]

_CACHE = {}
DEBUG = False


def _build(bias_on):
    nc = bacc.Bacc("TRN2", target_bir_lowering=False)

    vT_h = nc.dram_tensor("vT16", [D, N], BF16, kind="ExternalInput")
    qTin_h = nc.dram_tensor("qT16", [D, N], BF16, kind="ExternalInput")
    kTin_h = nc.dram_tensor("kT16", [D, N], BF16, kind="ExternalInput")
    wv_h = nc.dram_tensor("Wv16", [D, D], BF16, kind="ExternalInput")
    wq_h = nc.dram_tensor("Wq16", [D, D], BF16, kind="ExternalInput")
    wk_h = nc.dram_tensor("Wk16", [D, D], BF16, kind="ExternalInput")
    wo_h = nc.dram_tensor("Wo16", [D, D], BF16, kind="ExternalInput")
    ws1_h = nc.dram_tensor("Wvs18", [D, HID], F8, kind="ExternalInput")
    vT8_h = nc.dram_tensor("vT8", [D, N], F8, kind="ExternalInput")
    ws2_h = nc.dram_tensor("Ws28", [HID, D], F8, kind="ExternalInput")
    wc1_h = nc.dram_tensor("Wc1", [D, HID], BF16, kind="ExternalInput")
    wc2_h = nc.dram_tensor("Wc2", [HID, D], BF16, kind="ExternalInput")
    b_h = {}
    for nm in BIAS_NAMES:
        if bias_on[nm]:
            b_h[nm] = nc.dram_tensor(
                nm, [HID if nm in ("bs1", "bc1") else D], F32,
                kind="ExternalInput")
    out_h = nc.dram_tensor("outB", [N, D], BF16, kind="ExternalOutput")
    dbg_h = {}
    if DEBUG:
        for nm, cols in [("vvT", K4 * N), ("qTp", K4 * N), ("kTp", K4 * N),
                         ("vdE", T8 * D), ("outT", K4 * N)]:
            dbg_h[nm] = nc.dram_tensor("dbg_" + nm, [P, cols], BF16,
                                       kind="ExternalOutput")

    with tile.TileContext(nc) as tc:
        with tc.tile_pool(name="const", bufs=1) as cpool, \
             tc.tile_pool(name="wts", bufs=1) as wts, \
             tc.tile_pool(name="big", bufs=1) as big, \
             tc.tile_pool(name="swp", bufs=2) as swp, \
             tc.tile_pool(name="ptp", bufs=PV_LAG + 5) as ptp, \
             tc.tile_pool(name="atp", bufs=2) as atp, \
             tc.tile_pool(name="rcp", bufs=2) as rcp:
            psS = tc.alloc_tile_pool(name="psS", bufs=2, space="PSUM",
                                     side="right")
            # phase-1 only: second psum pool in the banks the attention
            # pools will use later; alternating projection groups between
            # psS/psB doubles the effective pipeline depth. Released before
            # the attention pools are allocated.
            psB = tc.alloc_tile_pool(name="psB", bufs=2, space="PSUM",
                                     side="left")

            # ACT queue: tiny dummy sigmoid first so the one table load
            # that covers sigmoid+relu+identity+copy happens at t=0; a
            # dummy exp is emitted later (after the last sigmoid) to swap
            # in the exp set before attention needs it.
            scr = cpool.tile([1, 1], F32, tag="scr")
            nc.scalar.activation(scr[:], scr[:], AF.Sigmoid)
            # the warmup transposes read a DVE-memset scratch (ready at
            # ~0.4us; DVE has no DMA queue), so id16 itself may land
            # whenever the Pool queue gets to it (first needed at ~22us)
            with tc.high_priority():
                wusrc = cpool.tile([P, P], BF16, tag="wusrc")
                nc.vector.memset(wusrc[:], 0.0)
                ones1 = cpool.tile([P, 1], BF16, tag="ones1")
                nc.vector.memset(ones1[:], 1.0)
                id16 = cpool.tile([P, P], BF16, tag="id16")
                make_identity(nc, id16[:])

            # ------- input DMAs spread across engine queues -------
            # Only SP/ACT/Pool can issue DMAs; a DMA occupies its issuing
            # queue for the whole transfer, so value-path tensors go first
            # on each queue.
            def ld(pool, name, handle, rows, cols, dt, eng, halves=False):
                t = pool.tile([P, (rows // P) * cols], dt, tag=name)
                src = handle[:].rearrange("(k p) c -> p k c", p=P)
                dst = t[:].rearrange("p (k c) -> p k c", k=rows // P)
                if halves:
                    hw = cols // 2
                    for j in range(2):
                        eng[j].dma_start(dst[:, :, j * hw:(j + 1) * hw],
                                         src[:, :, j * hw:(j + 1) * hw])
                else:
                    eng.dma_start(dst, src)
                return t

            wv16 = ld(wts, "wv16", wv_h, D, D, BF16, nc.sync)
            vT16 = ld(big, "vT16", vT_h, D, N, BF16,
                      [nc.scalar, nc.sync], halves=True)
            qT16 = ld(big, "qT16", qTin_h, D, N, BF16,
                      [nc.gpsimd, nc.gpsimd], halves=True)
            kT16 = ld(big, "kT16", kTin_h, D, N, BF16,
                      [nc.scalar, nc.gpsimd], halves=True)
            vT8 = ld(big, "vT8", vT8_h, D, N, F8, nc.scalar)
            wvs18 = ld(wts, "wvs18", ws1_h, D, HID, F8, nc.sync)
            ws28 = ld(wts, "ws28", ws2_h, HID, D, F8, nc.sync)
            wc1 = ld(wts, "wc1", wc1_h, D, HID, BF16, nc.sync)
            wc2 = ld(wts, "wc2", wc2_h, HID, D, BF16, nc.sync)
            wk16 = ld(wts, "wk16", wk_h, D, D, BF16, nc.sync)

            def load_bias(nm, rows):
                if not bias_on[nm]:
                    return [None] * (rows // P)
                nk = rows // P
                bt = cpool.tile([P, nk], F32, tag=nm)
                nc.sync.dma_start(
                    bt[:], b_h[nm][:].rearrange("(k p) -> p k", p=P))
                return [bt[:, k:k + 1] for k in range(nk)]

            bv_t = load_bias("bv", D)
            bs1_t = load_bias("bs1", HID)
            bs2_t = load_bias("bs2", D)
            bq_t = load_bias("bq", D)
            bk_t = load_bias("bk", D)
            bc1_t = load_bias("bc1", HID)
            bc2_t = load_bias("bc2", D)

            # ------- persistent tiles -------
            vvT = big.tile([P, K4 * N], BF16, tag="vvT")
            s1T = big.tile([P, M2 * N], F8, tag="s1T")
            qTp = big.tile([P, K4 * N], BF16, tag="qTp")
            kTp = big.tile([P, K4 * N], BF16, tag="kTp")
            vdE = big.tile([P, T8 * D], BF16, tag="vdE")
            outT = big.tile([P, K4 * N], BF16, tag="outT")
            stg = big.tile([P, T8 * D], BF16, tag="stg")
            wv2 = wts.tile([P, K4 * D], BF16, tag="wv2")

            # channel profile: mean over tokens of value, on DVE; emitted
            # interleaved between the vv projections so each reduce queues
            # BEHIND the vv psum moves (the mean is not needed until the
            # channel MLP at ~10us)
            profr = []

            def profile_k(k):
                pr = cpool.tile([P, 1], F32, tag=f"prof{k}")
                nc.vector.reduce_sum(pr[:], vT16[:, k * N:(k + 1) * N],
                                     axis=AX.X)
                prr = cpool.tile([P, 1], BF16, tag=f"profr{k}")
                nc.gpsimd.tensor_scalar_mul(prr[:], pr[:], 1.0 / N)
                profr.append(prr)
            wq16 = ld(wts, "wq16", wq_h, D, D, BF16, nc.gpsimd)
            wo16 = ld(wts, "wo16", wo_h, D, D, BF16, nc.gpsimd)

            ws28v = ws28[:].rearrange("p (k m) -> p k m", k=M2)
            s1Tv = s1T[:].rearrange("p (k t) -> p k t", k=M2)
            wvs18v = wvs18[:].rearrange("p (k m) -> p k m", k=K4)
            vT8v = vT8[:].rearrange("p (k t) -> p k t", k=K4)

            # alternate phase-1 psum groups between the two pools
            _pools = [psS, psB]
            _pi = [0]

            def ph_tile():
                pool = _pools[_pi[0] % len(_pools)]
                _pi[0] ^= 1
                ps = pool.tile([P, N], F32,
                               tag="psS" if pool is psS else "psB",
                               name=f"ph{_pi[0]}")
                return ps

            # PE p-state warmup: junk transposes while the first DMAs land
            wu = psS.tile([P, N], F32, tag="psS")
            wub = wu[:].bitcast(BF16)
            for i in range(WARMUP):
                nc.tensor.transpose(wub[:, (i % 4) * P:(i % 4 + 1) * P],
                                    wusrc[:], wusrc[:])

            # ------- value path -------
            def vv_m(m):
                ps = ph_tile()
                for half in range(2):
                    for k in range(K4):
                        nc.tensor.matmul(
                            ps[:, half * 512:(half + 1) * 512],
                            wv16[:, k * D + m * P:k * D + (m + 1) * P],
                            vT16[:, k * N + half * 512:k * N + half * 512 + 512],
                            start=(k == 0), stop=(k == K4 - 1))
                dst = vvT[:, m * N:(m + 1) * N]
                if m == 0:  # ACT takes only m0: the channel-MLP's tiny
                    # relu/sigmoid chain must slot into ACT right after
                    if bias_on["bv"]:
                        nc.scalar.activation(dst, ps[:], AF.Identity,
                                             bias=bv_t[m][:])
                    else:
                        nc.scalar.copy(dst, ps[:])
                else:
                    if bias_on["bv"]:
                        nc.vector.tensor_scalar(out=dst, in0=ps[:],
                                                scalar1=bv_t[m][:],
                                                op0=OP.add)
                    else:
                        nc.vector.tensor_copy(dst, ps[:])

            def s1_m(m2):
                # s1 = relu(value @ (Wv@Ws1) + b): the weight product is
                # folded on the host, so the matmul contracts the raw fp8
                # value with DoubleRow (two 256-deep passes)
                ps = ph_tile()
                for half in range(2):
                    for j in range(2):
                        nc.tensor.matmul(
                            ps[:, half * 512:(half + 1) * 512],
                            wvs18v[:, 2 * j:2 * j + 2,
                                   m2 * P:(m2 + 1) * P],
                            vT8v[:, 2 * j:2 * j + 2,
                                 half * 512:half * 512 + 512],
                            start=(j == 0), stop=(j == 1),
                            perf_mode=PM.DoubleRow)
                dst = s1T[:, m2 * N:(m2 + 1) * N]
                if m2 == 0:
                    nc.scalar.activation(dst, ps[:], AF.Relu,
                                         bias=bs1_t[m2][:] if bias_on["bs1"]
                                         else 0.0)
                else:  # relu on DVE: max(x + b, 0)
                    nc.vector.tensor_scalar(
                        out=dst, in0=ps[:],
                        scalar1=bs1_t[m2][:] if bias_on["bs1"] else 0.0,
                        scalar2=0.0, op0=OP.add, op1=OP.max)

            def sw_m(m):
                ps = ph_tile()
                for half in range(2):
                    nc.tensor.matmul(
                        ps[:, half * 512:(half + 1) * 512],
                        ws28v[:, :, m * P:(m + 1) * P],
                        s1Tv[:, :, half * 512:half * 512 + 512],
                        start=True, stop=True, perf_mode=PM.DoubleRow)
                sw = swp.tile([P, N], BF16, tag="swt")
                nc.scalar.activation(sw[:], ps[:], AF.Sigmoid,
                                     bias=bs2_t[m][:] if bias_on["bs2"]
                                     else 0.0)
                sw_m.last = sw
                nc.gpsimd.tensor_tensor(out=vvT[:, m * N:(m + 1) * N],
                                        in0=vvT[:, m * N:(m + 1) * N],
                                        in1=sw[:], op=OP.mult)

            def vch_m(m):
                # channel-gated value projection: cw is folded into wv2
                ps = ph_tile()
                for half in range(2):
                    for k in range(K4):
                        nc.tensor.matmul(
                            ps[:, half * 512:(half + 1) * 512],
                            wv2[:, k * D + m * P:k * D + (m + 1) * P],
                            vT16[:, k * N + half * 512:k * N + half * 512 + 512],
                            start=(k == 0), stop=(k == K4 - 1))
                sl = slice(m * N, (m + 1) * N)
                if bias_on["bv"]:
                    nc.vector.scalar_tensor_tensor(
                        out=vvT[:, sl], in0=ps[:], scalar=bv_t[m][:],
                        in1=vvT[:, sl], op0=OP.add, op1=OP.add)
                else:
                    nc.vector.tensor_tensor(out=vvT[:, sl], in0=ps[:],
                                            in1=vvT[:, sl], op=OP.add)

            vd_ps = [None]

            def vd_tr(t8):
                # two vd transposes share one psS slot via its bf16 view
                if t8 % 4 == 0:
                    ps = ph_tile()
                    vd_ps[0] = ps
                psb = vd_ps[0][:].bitcast(BF16)
                base = (t8 % 4) * D
                for k in range(K4):
                    nc.tensor.transpose(
                        psb[:, base + k * P:base + (k + 1) * P],
                        vvT[:, k * N + t8 * P:k * N + (t8 + 1) * P],
                        id16[:])
                if t8 % 2 == 0:
                    nc.vector.tensor_copy(vdE[:, t8 * D:(t8 + 1) * D],
                                          psb[:, base:base + D])
                else:
                    nc.scalar.copy(vdE[:, t8 * D:(t8 + 1) * D],
                                   psb[:, base:base + D])

            def proj_m(wt, bt, has_b, xt, OUT, m, move_eng):
                ps = ph_tile()
                for half in range(2):
                    for k in range(K4):
                        nc.tensor.matmul(
                            ps[:, half * 512:(half + 1) * 512],
                            wt[:, k * D + m * P:k * D + (m + 1) * P],
                            xt[:, k * N + half * 512:k * N + half * 512 + 512],
                            start=(k == 0), stop=(k == K4 - 1))
                dst = OUT[:, m * N:(m + 1) * N]
                if move_eng == "act":
                    if has_b:
                        nc.scalar.activation(dst, ps[:], AF.Identity,
                                             bias=bt[m][:])
                    else:
                        nc.scalar.copy(dst, ps[:])
                else:
                    if has_b:
                        nc.vector.tensor_scalar(out=dst, in0=ps[:],
                                                scalar1=bt[m][:], op0=OP.add)
                    else:
                        nc.vector.tensor_copy(dst, ps[:])

            for k in range(K4):
                profile_k(k)
            for m in range(K4):
                vv_m(m)
            # channel MLP, both stages in one shared psum slot
            ps_c = ph_tile()
            c1r = []
            for m2 in range(M2):
                for k in range(K4):
                    nc.tensor.matmul(
                        ps_c[:, m2:m2 + 1],
                        wc1[:, k * HID + m2 * P:k * HID + m2 * P + P],
                        profr[k][:], start=(k == 0), stop=(k == K4 - 1))
                cr = cpool.tile([P, 1], BF16, tag=f"c1r{m2}")
                nc.scalar.activation(cr[:], ps_c[:, m2:m2 + 1], AF.Relu,
                                     bias=bc1_t[m2][:] if bias_on["bc1"]
                                     else 0.0)
                c1r.append(cr)
            cw_t = []
            for m in range(K4):
                for k in range(M2):
                    nc.tensor.matmul(
                        ps_c[:, 4 + m:5 + m],
                        wc2[:, k * D + m * P:k * D + m * P + P],
                        c1r[k][:], start=(k == 0), stop=(k == M2 - 1))
                cw = cpool.tile([P, 1], F32, tag=f"cw{m}")
                nc.scalar.activation(cw[:], ps_c[:, 4 + m:5 + m], AF.Sigmoid,
                                     bias=bc2_t[m][:] if bias_on["bc2"]
                                     else 0.0)
                cw_t.append(cw)
            # fold the channel gate into the value weight (per input row)
            for k in range(K4):
                eng = nc.gpsimd if k % 2 == 0 else nc.vector
                eng.tensor_scalar_mul(out=wv2[:, k * D:(k + 1) * D],
                                      in0=wv16[:, k * D:(k + 1) * D],
                                      scalar1=cw_t[k][:])
            for m2 in range(M2):
                s1_m(m2)
            # interleave gating with the channel-path projection
            sw_m(0)
            vch_m(0)
            sw_m(1)
            vch_m(1)
            sw_m(2)
            vch_m(2)
            sw_m(3)
            vch_m(3)
            # swap the ACT tables to the exp set now (last sigmoid is done);
            # identity/relu/copy live in both sets so value-phase moves
            # after this point still need no load. Reading the last sw tile
            # pins the dummy BEHIND the final sigmoid -- without the data
            # dependency the scheduler hoists it and thrashes the tables.
            nc.scalar.activation(scr[:], sw_m.last[0:1, 0:1], AF.Exp)
            proj_m(wk16, bk_t, bias_on["bk"], kT16, kTp, 0, "dve")
            proj_m(wq16, bq_t, bias_on["bq"], qT16, qTp, 0, "act")
            for t8 in range(T8):
                vd_tr(t8)
            def proj_half(wt, bt, has_b, xt, OUT, m, half, eng):
                # half-projection through the psT bank: paced through the
                # attention stream so each position carries ~850ns of PE
                # work and the 2-deep scores ring never outruns the exps
                ps = psT[0][:]
                base = half * 512
                for k in range(K4):
                    nc.tensor.matmul(
                        ps,
                        wt[:, k * D + m * P:k * D + (m + 1) * P],
                        xt[:, k * N + base:k * N + base + 512],
                        start=(k == 0), stop=(k == K4 - 1),
                        skip_group_check=True)
                dst = OUT[:, m * N + base:m * N + base + 512]
                if eng == "act":
                    if has_b:
                        nc.scalar.activation(dst, ps, AF.Identity,
                                             bias=bt[m][:])
                    else:
                        nc.scalar.copy(dst, ps)
                else:
                    if has_b:
                        nc.vector.tensor_scalar(out=dst, in0=ps,
                                                scalar1=bt[m][:],
                                                scalar2=None, op0=OP.add)
                    else:
                        nc.vector.tensor_copy(dst, ps)

            fillers = {
                0: lambda: proj_m(wk16, bk_t, bias_on["bk"], kT16, kTp,
                                  1, "act"),
                2: lambda: proj_m(wq16, bq_t, bias_on["bq"], qT16, qTp,
                                  1, "dve"),
                4: lambda: proj_m(wk16, bk_t, bias_on["bk"], kT16, kTp,
                                  3, "act"),
            }
            for pos, (wt_, bt_, bn, xt_, OUT_, m_, eng_) in [
                (8, (wk16, bk_t, "bk", kT16, kTp, 2, "dve")),
                (14, (wk16, bk_t, "bk", kT16, kTp, 2, "dve")),
                (20, (wq16, bq_t, "bq", qT16, qTp, 2, "dve")),
                (26, (wq16, bq_t, "bq", qT16, qTp, 2, "dve")),
                (32, (wq16, bq_t, "bq", qT16, qTp, 3, "dve")),
                (38, (wq16, bq_t, "bq", qT16, qTp, 3, "dve")),
            ]:
                half_ = 0 if pos in (8, 20, 32) else 1
                fillers[pos] = (
                    lambda w=wt_, b=bt_, bn_=bn, x=xt_, O=OUT_, m=m_,
                    hf=half_, e=eng_:
                    proj_half(w, b, bias_on[bn_], x, O, m, hf, e))

            # ------- attention -------
            boB = None
            if bias_on["bo"]:
                boB = cpool.tile([P, D], F32, tag="boB")
                nc.sync.dma_start(boB[:],
                                  b_h["bo"][None, :].to_broadcast((P, D)))

            def scores_chunk(h, k8):
                p0 = (h % 2) * DH
                cc = (h // 2) * N
                ps = psS.tile([P, N], F32, tag="psS")
                for half in range(2):
                    nc.tensor.matmul(
                        ps[:, half * 512:(half + 1) * 512],
                        kTp[p0:p0 + DH, cc + k8 * P:cc + (k8 + 1) * P],
                        qTp[p0:p0 + DH,
                            cc + half * 512:cc + half * 512 + 512],
                        start=True, stop=True)
                pt = ptp.tile([P, N], BF16, tag="pt")
                if k8 in DVE_K8:
                    nc.vector.tensor_scalar(
                        out=pt[:].bitcast(I16), in0=ps[:], scalar1=SCH_K1,
                        scalar2=SCH_K2, op0=OP.mult, op1=OP.add)
                else:
                    nc.scalar.activation(pt[:], ps[:], AF.Exp, scale=0.125)
                return pt

            psD = [None]  # denominator bank, allocated at stream start
            psT = [None]  # pair-transpose staging bank

            def pv_chunk(h, k8, G, pt):
                # out[q, d] orientation: stationary = probability chunk,
                # moving = v; each ap=1 ones-matmul accumulates the softmax
                # denominator for the same q-chunk in the den bank.
                mv = vdE[:, k8 * D + h * DH:k8 * D + (h + 1) * DH]
                for q8 in range(8):
                    st = pt[:, q8 * P:(q8 + 1) * P]
                    # start=True clears the WHOLE bank's has_written bits,
                    # so only the first write of the pass may carry it; the
                    # other slots write into cleared bits (write-then-set)
                    nc.tensor.matmul(
                        G[:, q8 * DH:(q8 + 1) * DH], st, mv,
                        start=(k8 == 0 and q8 == 0), stop=(k8 == T8 - 1),
                        skip_group_check=True)
                    nc.tensor.matmul(
                        psD[0][:, h * 8 + q8:h * 8 + q8 + 1], st, ones1[:],
                        start=(k8 == 0 and q8 == 0), stop=(k8 == T8 - 1),
                        skip_group_check=True)

            rec8s = {}
            attPs = {}
            gRaws = {}

            def norm_recip(h, G):
                rec8 = rcp.tile([P, 8], F32, tag="rec8", name="rec8")
                nc.vector.reciprocal(rec8[:], psD[0][:, h * 8:(h + 1) * 8])
                rec8s[h] = rec8
                # un-normalized numerators to SBUF so GpSimd (which cannot
                # read PSUM) can do the normalize multiplies; the last head
                # is on the critical tail path, so split it across engines
                gr = rcp.tile([P, 512], BF16, tag="gRaw", name="gRaw")
                if h == H - 1:
                    nc.vector.tensor_copy(gr[:, 0:256], G[:, 0:256])
                    nc.scalar.copy(gr[:, 256:512], G[:, 256:512])
                else:
                    nc.vector.tensor_copy(gr[:], G[:])
                gRaws[h] = gr

            def norm_mult(h, G, q8s, eng):
                # normalized bf16 [token, d] staging for the pair transpose
                if h % 2 == 0 and h not in attPs:
                    attPs[h] = atp.tile([P, T8 * P], BF16, tag="attP", name="attP")
                aP = attPs[h - h % 2]
                rec8 = rec8s[h]
                gr = gRaws[h]
                for q8 in q8s:
                    dst = aP[:, q8 * P + (h % 2) * DH:
                             q8 * P + (h % 2) * DH + DH]
                    nc.gpsimd.tensor_scalar_mul(
                        out=dst, in0=gr[:, q8 * DH:(q8 + 1) * DH],
                        scalar1=rec8[:, q8:q8 + 1])

            def pair_transpose(j):
                aP = attPs[2 * j]
                psTb = psT[0][:].bitcast(BF16)
                for q8 in range(8):
                    nc.tensor.transpose(psTb[:, q8 * P:(q8 + 1) * P],
                                        aP[:, q8 * P:(q8 + 1) * P], id16[:])

            def pair_copy(j, eng):
                psTb = psT[0][:].bitcast(BF16)
                if eng == "act":
                    nc.scalar.copy(outT[:, j * N:(j + 1) * N], psTb[:, 0:N])
                else:
                    nc.vector.tensor_copy(outT[:, j * N:(j + 1) * N],
                                          psTb[:, 0:N])

            acc = big.tile([P, T8 * D], BF16, tag="acc")

            def partial012(q8):
                # out[q8] partials: k=0,1,2 chunks of outT.T @ Wo. While
                # scores still run, use the psT bank so they keep both psS
                # buffers; the last two partials run after the final scores
                # chunk and take a psS tile (two groups in one tile),
                # breaking the psT write->copy->write serialization.
                if q8 < 6:
                    ps = psT[0][:]
                elif q8 % 2 == 0:
                    partial012.ps6 = psS.tile([P, N], F32, tag="psS")
                    ps = partial012.ps6[:, 0:D]
                else:
                    ps = partial012.ps6[:, D:2 * D]
                for k in range(3):
                    nc.tensor.matmul(
                        ps,
                        outT[:, k * N + q8 * P:k * N + (q8 + 1) * P],
                        wo16[:, k * D:(k + 1) * D],
                        start=(k == 0), stop=(k == 2),
                        skip_group_check=True)
                at = acc[:, q8 * D:(q8 + 1) * D]
                if bias_on["bo"]:
                    nc.vector.tensor_tensor(out=at, in0=ps, in1=boB[:],
                                            op=OP.add)
                else:
                    nc.vector.tensor_copy(at, ps)

            # flat software pipeline across all heads: a continuous stream
            # of scores->exp chunks with PV trailing PV_LAG chunks behind.
            TOT = H * T8
            pts = [None] * TOT
            Gs = [None] * H
            # first position strictly after pair 2's outT copy (head 5
            # done at 8*5+7+PV_LAG, copy 5 positions later)
            p012_base = 8 * 5 + 7 + PV_LAG + 6
            for q8 in range(8):
                fillers[p012_base + q8] = (lambda qq=q8: partial012(qq))
            recip_at = {}
            mult_at = {}
            transp_at = {}
            copy_at = {}
            psG = None
            for i in range(TOT + PV_LAG + 10):
                # norm/transpose/copy events first: several share the psT
                # bank with the half-projection fillers, and emission order
                # is what Tile's dependency tracking sees
                if i in recip_at:
                    norm_recip(recip_at[i], Gs[recip_at[i]])
                for key in (i, i + 0.5, i + 0.75):
                    if key in mult_at:
                        hh, q8s, eng = mult_at[key]
                        norm_mult(hh, Gs[hh], q8s, eng)
                if i in transp_at:
                    pair_transpose(transp_at[i])
                if i in copy_at:
                    pair_copy(copy_at[i], "dve")
                if i in fillers:
                    fillers[i]()
                j = i - PV_LAG
                if j == 0:
                    # all psB users (value-path/projection fillers) are done;
                    # hand its banks to the attention pools
                    psB.release()
                    _pools[:] = [psS]
                    _pi[0] = 0
                    psG = tc.alloc_tile_pool(name="psG", bufs=2,
                                             space="PSUM", side="left")
                    psDp = tc.alloc_tile_pool(name="psD", bufs=1,
                                              space="PSUM", side="left")
                    psTp = tc.alloc_tile_pool(name="psT", bufs=1,
                                              space="PSUM", side="left")
                    psD[0] = psDp.tile([P, 512], F32, tag="psD", name="psDt")
                    psT[0] = psTp.tile([P, 512], F32, tag="psT", name="psTt")
                if 0 <= j < TOT:
                    h, k8 = divmod(j, T8)
                    if k8 == 0:
                        Gs[h] = psG.tile([P, 512], F32, tag="G", name="Gt")
                    # PV emitted before this position's scores: its pt is
                    # long ready, and the scores' wait on the psS ring then
                    # lands ~220ns later, giving the exps more deadline
                    pv_chunk(h, k8, Gs[h], pts[j])
                    pts[j] = None
                    if k8 == T8 - 1 and h < H - 1:
                        recip_at[i + 1] = h
                        mult_at[i + 2] = (h, range(0, 4), "pool")
                        mult_at[i + 3] = (h, range(4, 8), "pool")
                        if h % 2 == 1:
                            transp_at[i + 4] = h // 2
                            copy_at[i + 5] = h // 2
                if i < TOT:
                    h, k8 = divmod(i, T8)
                    pts[i] = scores_chunk(h, k8)

            # ------- tail: last head's norm, pair 3, k=3 partials, out -----
            h7 = H - 1
            norm_recip(h7, Gs[h7])
            psTb = psT[0][:].bitcast(BF16)
            aP3 = attPs[6]
            # interleave: 4 mults -> 4 transposes -> first outT half-copy,
            # so the first k3 matmuls start while the second half is still
            # normalizing
            norm_mult(h7, Gs[h7], range(0, 4), "pool")
            for q8 in range(4):
                nc.tensor.transpose(psTb[:, q8 * P:(q8 + 1) * P],
                                    aP3[:, q8 * P:(q8 + 1) * P], id16[:])
            nc.vector.tensor_copy(outT[:, 3 * N:3 * N + 512], psTb[:, 0:512])
            norm_mult(h7, Gs[h7], range(4, 8), "pool")
            for q8 in range(4, 8):
                nc.tensor.transpose(psTb[:, q8 * P:(q8 + 1) * P],
                                    aP3[:, q8 * P:(q8 + 1) * P], id16[:])
            nc.scalar.copy(outT[:, 3 * N + 512:4 * N], psTb[:, 512:N])

            def tail_q8(q8):
                # per-q8 output group: k3 matmul + identity-matmul fold of
                # the k=0..2 partial (acc) on the idle PE, then a plain
                # copy (ACT/DVE alternating) and its own DMA (SP/Pool
                # alternating). Rotates psG halves + psS halves: a 6-slot
                # ring, so no group ever waits on a copy.
                if q8 % 4 < 2:
                    g = psG.tile([P, 512], F32, tag="G", name="Gt")
                    ps = g[:]
                else:
                    if q8 % 4 == 2:
                        tail_q8.ps = psS.tile([P, N], F32, tag="psS")
                    ps = tail_q8.ps[:, (q8 % 4 - 2) * D:(q8 % 4 - 1) * D]
                nc.tensor.matmul(
                    ps,
                    outT[:, 3 * N + q8 * P:3 * N + (q8 + 1) * P],
                    wo16[:, 3 * D:4 * D],
                    start=True, stop=False, skip_group_check=True)
                nc.tensor.matmul(
                    ps, id16[:], acc[:, q8 * D:(q8 + 1) * D],
                    start=False, stop=True, skip_group_check=True)
                sl = slice(q8 * D, (q8 + 1) * D)
                if q8 % 2 == 0:
                    nc.vector.tensor_copy(stg[:, sl], ps)
                    dma = nc.scalar
                else:
                    nc.scalar.copy(stg[:, sl], ps)
                    dma = nc.sync
                dma.dma_start(out_h[q8 * P:(q8 + 1) * P, :], stg[:, sl])

            for q8 in range(8):
                tail_q8(q8)
            if DEBUG:
                for nm, t in [("vvT", vvT), ("qTp", qTp), ("kTp", kTp),
                              ("vdE", vdE), ("outT", outT)]:
                    nc.sync.dma_start(dbg_h[nm][:], t[:])
            psS.release()
            psTp.release()
            psDp.release()
            psG.release()

    nc.finalize()
    return nc


def get_nc(bias_key=frozenset()):
    if bias_key not in _CACHE:
        bias_on = {nm: (nm in bias_key) for nm in BIAS_NAMES}
        _CACHE[bias_key] = _build(bias_on)
    return _CACHE[bias_key]


def make_in_maps(inputs):
    """Host-side marshalling: transpose + quantize, shard over batch."""
    if "key_in" not in inputs and "key" in inputs:
        inputs = dict(inputs)
        inputs["key_in"] = inputs.pop("key")
    f32 = np.float32
    bs1_eff = (np.asarray(inputs["bs1"], np.float64)
               + np.asarray(inputs["bv"], np.float64)
               @ np.asarray(inputs["Ws1"], np.float64)).astype(f32)
    inputs = dict(inputs)
    inputs["bs1"] = bs1_eff
    bias_key = frozenset(
        nm for nm in BIAS_NAMES
        if np.any(np.asarray(inputs[nm], f32) != 0.0))
    shared = {
        "Wv16": np.asarray(inputs["Wv"], f32).astype(BF16NP),
        "Wq16": np.asarray(inputs["Wq"], f32).astype(BF16NP),
        "Wk16": np.asarray(inputs["Wk"], f32).astype(BF16NP),
        "Wo16": np.asarray(inputs["Wo"], f32).astype(BF16NP),
        "Wvs18": (np.asarray(inputs["Wv"], np.float64)
                  @ np.asarray(inputs["Ws1"], np.float64)).astype(F8NP),
        "Ws28": np.asarray(inputs["Ws2"], f32).astype(F8NP),
        "Wc1": np.asarray(inputs["Wc1"], f32).astype(BF16NP),
        "Wc2": np.asarray(inputs["Wc2"], f32).astype(BF16NP),
    }
    for nm in bias_key:
        shared[nm] = np.ascontiguousarray(np.asarray(inputs[nm], f32))
    q = np.asarray(inputs["query"], f32)
    k = np.asarray(inputs["key_in"], f32)
    v = np.asarray(inputs["value"], f32)
    in_maps = []
    for c in range(NCORES):
        m = dict(shared)
        m["vT16"] = np.ascontiguousarray(v[c].T).astype(BF16NP)
        m["vT8"] = np.ascontiguousarray(v[c].T).astype(F8NP)
        m["qT16"] = np.ascontiguousarray(q[c].T).astype(BF16NP)
        m["kT16"] = np.ascontiguousarray(k[c].T).astype(BF16NP)
        in_maps.append(m)
    return in_maps, bias_key


def kernel(**inputs):
    in_maps, bias_key = make_in_maps(inputs)
    nc = get_nc(bias_key)
    res = run_bass_kernel_spmd(nc, in_maps, core_ids=list(range(NCORES)))
    return np.stack(
        [res.results[c]["outB"].astype(np.float32) for c in range(NCORES)],
        axis=0)


# revision 56
# speedup vs baseline: 1.0117x; 1.0117x over previous
"""DualAttention Trainium2 kernel: 8-core data-parallel over batch.

Each NeuronCore processes one batch element [1024, 512]. Host-side
marshalling transposes the three input tensors to [d, token] layout and
quantizes to bf16 (fp8 where the error budget allows: the spatial-gate
MLP runs fp8 DoubleRow end to end, with Wv@Ws1 folded on the host so
its first matmul contracts the raw fp8 value in two 256-deep passes).

Input DMAs are spread across the SP/ACT/Pool queues (a DMA occupies its
issuing queue for the whole transfer), value-path tensors first, so all
inputs land ~4x sooner than a single-queue cascade. ACT's queue opens
with a dummy sigmoid and swaps to the exp table set via a dummy exp
pinned (by a data dependency) behind the last sigmoid: two table loads
total, both off the critical path.

All activations stay "transposed" (d on partitions, tokens free) so
every projection contracts over the partition dim; attention scores are
computed transposed ([k_tok, q_tok]). PV uses the probability chunk as
the stationary operand and v as the moving operand, producing [q_tok,
d] blocks: 64-column streams fill all 128 output partitions, halving
PV's PE time vs the [d, q] orientation. Softmax denominators accumulate
in a dedicated PSUM bank via ap=1 ones-column matmuls that reuse the
loaded stationary (~free). PSUM start=True clears the WHOLE bank's
has_written bits, so only the first write of each accumulation pass
carries it. Normalization is a per-partition reciprocal, one bf16 copy
of the numerators to SBUF, and GpSimd scalar-multiplies (GpSimd cannot
touch PSUM); head pairs are then PE-transposed back to [d, token] for
the output projection.

The attention stream is a flat software pipeline: per position, PV
(trailing PV_LAG chunks), one scores chunk, and paced filler work --
half-projections of the q/k weights through the single-bank psT pool
and k<=2 output-projection partials -- sized so ~2 positions of PE work
cover the exp turnaround of the 2-deep scores ring. Exps run on ACT
except three Schraudolph chunks per head on DVE (bf16 bits via int16
truncation). The tail folds the k=0..2 partials into each per-q8 k3
group with an identity matmul on the idle PE, so the final moves are
plain copies alternating ACT/DVE, each followed by its own DMA on the
opposite queue.
"""
import numpy as np
import ml_dtypes

import concourse.bass as bass
import concourse.tile as tile
from concourse import bacc, mybir
from concourse.bass_utils import run_bass_kernel_spmd
from concourse.masks import make_identity

B, N, D = 8, 1024, 512
H, DH, HID = 8, 64, 256
P = 128
K4 = D // P          # 4 d-chunks
T8 = N // P          # 8 token chunks
M2 = HID // P        # 2 hidden chunks
NCORES = 8
PV_LAG = 6
WARMUP = 0          # dummy transposes to ramp the PE p-state

F32 = mybir.dt.float32
F8 = mybir.dt.float8e4
BF16 = mybir.dt.bfloat16
I16 = mybir.dt.int16
AF = mybir.ActivationFunctionType
OP = mybir.AluOpType
AX = mybir.AxisListType
PM = mybir.MatmulPerfMode

F8NP = ml_dtypes.float8_e4m3
BF16NP = ml_dtypes.bfloat16

# Schraudolph exp in bf16 bits: i16 = trunc(x*0.125*log2e*128 + K2)
LOG2E = 1.4426950408889634
SCH_K1 = 0.125 * LOG2E * 128.0
SCH_K2 = 127.0 * 128.0 - 7.0     # -7 centers the multiplicative bias

# k8 positions per head whose exp runs as Schraudolph on DVE (GpSimd
# cannot touch PSUM, so the scores psum is only reachable from ACT/DVE)
DVE_K8 = (1, 4, 6)

WEIGHT_NAMES = ["Wq", "Wk", "Wv", "Wo", "Ws1", "Ws2", "Wc1", "Wc2"]
BIAS_NAMES = ["bq", "bk", "bv", "bo", "bs1", "bs2", "bc1", "bc2"]

_CACHE = {}
DEBUG = False


def _build(bias_on):
    nc = bacc.Bacc("TRN2", target_bir_lowering=False)

    vT_h = nc.dram_tensor("vT16", [D, N], BF16, kind="ExternalInput")
    qTin_h = nc.dram_tensor("qT16", [D, N], BF16, kind="ExternalInput")
    kTin_h = nc.dram_tensor("kT16", [D, N], BF16, kind="ExternalInput")
    wv_h = nc.dram_tensor("Wv16", [D, D], BF16, kind="ExternalInput")
    wq_h = nc.dram_tensor("Wq16", [D, D], BF16, kind="ExternalInput")
    wk_h = nc.dram_tensor("Wk16", [D, D], BF16, kind="ExternalInput")
    wo_h = nc.dram_tensor("Wo16", [D, D], BF16, kind="ExternalInput")
    ws1_h = nc.dram_tensor("Wvs18", [D, HID], F8, kind="ExternalInput")
    vT8_h = nc.dram_tensor("vT8", [D, N], F8, kind="ExternalInput")
    ws2_h = nc.dram_tensor("Ws28", [HID, D], F8, kind="ExternalInput")
    wc1_h = nc.dram_tensor("Wc1", [D, HID], BF16, kind="ExternalInput")
    wc2_h = nc.dram_tensor("Wc2", [HID, D], BF16, kind="ExternalInput")
    b_h = {}
    for nm in BIAS_NAMES:
        if bias_on[nm]:
            b_h[nm] = nc.dram_tensor(
                nm, [HID if nm in ("bs1", "bc1") else D], F32,
                kind="ExternalInput")
    out_h = nc.dram_tensor("outB", [N, D], BF16, kind="ExternalOutput")
    dbg_h = {}
    if DEBUG:
        for nm, cols in [("vvT", K4 * N), ("qTp", K4 * N), ("kTp", K4 * N),
                         ("vdE", T8 * D), ("outT", K4 * N)]:
            dbg_h[nm] = nc.dram_tensor("dbg_" + nm, [P, cols], BF16,
                                       kind="ExternalOutput")

    with tile.TileContext(nc) as tc:
        with tc.tile_pool(name="const", bufs=1) as cpool, \
             tc.tile_pool(name="wts", bufs=1) as wts, \
             tc.tile_pool(name="big", bufs=1) as big, \
             tc.tile_pool(name="swp", bufs=2) as swp, \
             tc.tile_pool(name="ptp", bufs=PV_LAG + 5) as ptp, \
             tc.tile_pool(name="atp", bufs=2) as atp, \
             tc.tile_pool(name="rcp", bufs=2) as rcp:
            psS = tc.alloc_tile_pool(name="psS", bufs=2, space="PSUM",
                                     side="right")
            # phase-1 only: second psum pool in the banks the attention
            # pools will use later; alternating projection groups between
            # psS/psB doubles the effective pipeline depth. Released before
            # the attention pools are allocated.
            psB = tc.alloc_tile_pool(name="psB", bufs=2, space="PSUM",
                                     side="left")

            # ACT queue: tiny dummy sigmoid first so the one table load
            # that covers sigmoid+relu+identity+copy happens at t=0; a
            # dummy exp is emitted later (after the last sigmoid) to swap
            # in the exp set before attention needs it.
            scr = cpool.tile([1, 1], F32, tag="scr")
            nc.scalar.activation(scr[:], scr[:], AF.Sigmoid)
            # the warmup transposes read a DVE-memset scratch (ready at
            # ~0.4us; DVE has no DMA queue), so id16 itself may land
            # whenever the Pool queue gets to it (first needed at ~22us)
            with tc.high_priority():
                wusrc = cpool.tile([P, P], BF16, tag="wusrc")
                nc.vector.memset(wusrc[:], 0.0)
                ones1 = cpool.tile([P, 1], BF16, tag="ones1")
                nc.vector.memset(ones1[:], 1.0)
                id16 = cpool.tile([P, P], BF16, tag="id16")
                make_identity(nc, id16[:])

            # ------- input DMAs spread across engine queues -------
            # Only SP/ACT/Pool can issue DMAs; a DMA occupies its issuing
            # queue for the whole transfer, so value-path tensors go first
            # on each queue.
            def ld(pool, name, handle, rows, cols, dt, eng, halves=False):
                t = pool.tile([P, (rows // P) * cols], dt, tag=name)
                src = handle[:].rearrange("(k p) c -> p k c", p=P)
                dst = t[:].rearrange("p (k c) -> p k c", k=rows // P)
                if halves:
                    hw = cols // 2
                    for j in range(2):
                        eng[j].dma_start(dst[:, :, j * hw:(j + 1) * hw],
                                         src[:, :, j * hw:(j + 1) * hw])
                else:
                    eng.dma_start(dst, src)
                return t

            wv16 = ld(wts, "wv16", wv_h, D, D, BF16, nc.sync)
            vT16 = ld(big, "vT16", vT_h, D, N, BF16,
                      [nc.scalar, nc.sync], halves=True)
            qT16 = ld(big, "qT16", qTin_h, D, N, BF16,
                      [nc.gpsimd, nc.gpsimd], halves=True)
            kT16 = ld(big, "kT16", kTin_h, D, N, BF16,
                      [nc.scalar, nc.gpsimd], halves=True)
            vT8 = ld(big, "vT8", vT8_h, D, N, F8, nc.scalar)
            wvs18 = ld(wts, "wvs18", ws1_h, D, HID, F8, nc.sync)
            ws28 = ld(wts, "ws28", ws2_h, HID, D, F8, nc.sync)
            wc1 = ld(wts, "wc1", wc1_h, D, HID, BF16, nc.sync)
            wc2 = ld(wts, "wc2", wc2_h, HID, D, BF16, nc.sync)
            wk16 = ld(wts, "wk16", wk_h, D, D, BF16, nc.sync)

            def load_bias(nm, rows):
                if not bias_on[nm]:
                    return [None] * (rows // P)
                nk = rows // P
                bt = cpool.tile([P, nk], F32, tag=nm)
                nc.sync.dma_start(
                    bt[:], b_h[nm][:].rearrange("(k p) -> p k", p=P))
                return [bt[:, k:k + 1] for k in range(nk)]

            bv_t = load_bias("bv", D)
            bs1_t = load_bias("bs1", HID)
            bs2_t = load_bias("bs2", D)
            bq_t = load_bias("bq", D)
            bk_t = load_bias("bk", D)
            bc1_t = load_bias("bc1", HID)
            bc2_t = load_bias("bc2", D)

            # ------- persistent tiles -------
            vvT = big.tile([P, K4 * N], BF16, tag="vvT")
            s1T = big.tile([P, M2 * N], F8, tag="s1T")
            qTp = big.tile([P, K4 * N], BF16, tag="qTp")
            kTp = big.tile([P, K4 * N], BF16, tag="kTp")
            vdE = big.tile([P, T8 * D], BF16, tag="vdE")
            outT = big.tile([P, K4 * N], BF16, tag="outT")
            stg = big.tile([P, T8 * D], BF16, tag="stg")
            wv2 = wts.tile([P, K4 * D], BF16, tag="wv2")

            # channel profile: mean over tokens of value, on DVE; emitted
            # interleaved between the vv projections so each reduce queues
            # BEHIND the vv psum moves (the mean is not needed until the
            # channel MLP at ~10us)
            profr = []

            def profile_k(k):
                pr = cpool.tile([P, 1], F32, tag=f"prof{k}")
                nc.vector.reduce_sum(pr[:], vT16[:, k * N:(k + 1) * N],
                                     axis=AX.X)
                prr = cpool.tile([P, 1], BF16, tag=f"profr{k}")
                nc.gpsimd.tensor_scalar_mul(prr[:], pr[:], 1.0 / N)
                profr.append(prr)
            wq16 = ld(wts, "wq16", wq_h, D, D, BF16, nc.gpsimd)
            wo16 = ld(wts, "wo16", wo_h, D, D, BF16, nc.gpsimd)

            ws28v = ws28[:].rearrange("p (k m) -> p k m", k=M2)
            s1Tv = s1T[:].rearrange("p (k t) -> p k t", k=M2)
            wvs18v = wvs18[:].rearrange("p (k m) -> p k m", k=K4)
            vT8v = vT8[:].rearrange("p (k t) -> p k t", k=K4)

            # alternate phase-1 psum groups between the two pools
            _pools = [psS, psB]
            _pi = [0]

            def ph_tile():
                pool = _pools[_pi[0] % len(_pools)]
                _pi[0] ^= 1
                ps = pool.tile([P, N], F32,
                               tag="psS" if pool is psS else "psB",
                               name=f"ph{_pi[0]}")
                return ps

            # PE p-state warmup: junk transposes while the first DMAs land
            wu = psS.tile([P, N], F32, tag="psS")
            wub = wu[:].bitcast(BF16)
            for i in range(WARMUP):
                nc.tensor.transpose(wub[:, (i % 4) * P:(i % 4 + 1) * P],
                                    wusrc[:], wusrc[:])

            # ------- value path -------
            def vv_m(m):
                ps = ph_tile()
                for half in range(2):
                    for k in range(K4):
                        nc.tensor.matmul(
                            ps[:, half * 512:(half + 1) * 512],
                            wv16[:, k * D + m * P:k * D + (m + 1) * P],
                            vT16[:, k * N + half * 512:k * N + half * 512 + 512],
                            start=(k == 0), stop=(k == K4 - 1))
                dst = vvT[:, m * N:(m + 1) * N]
                if m % 2 == 0:  # alternate ACT/DVE so moves run in parallel
                    if bias_on["bv"]:
                        nc.scalar.activation(dst, ps[:], AF.Identity,
                                             bias=bv_t[m][:])
                    else:
                        nc.scalar.copy(dst, ps[:])
                else:
                    if bias_on["bv"]:
                        nc.vector.tensor_scalar(out=dst, in0=ps[:],
                                                scalar1=bv_t[m][:],
                                                op0=OP.add)
                    else:
                        nc.vector.tensor_copy(dst, ps[:])

            def s1_m(m2):
                # s1 = relu(value @ (Wv@Ws1) + b): the weight product is
                # folded on the host, so the matmul contracts the raw fp8
                # value with DoubleRow (two 256-deep passes)
                ps = ph_tile()
                for half in range(2):
                    for j in range(2):
                        nc.tensor.matmul(
                            ps[:, half * 512:(half + 1) * 512],
                            wvs18v[:, 2 * j:2 * j + 2,
                                   m2 * P:(m2 + 1) * P],
                            vT8v[:, 2 * j:2 * j + 2,
                                 half * 512:half * 512 + 512],
                            start=(j == 0), stop=(j == 1),
                            perf_mode=PM.DoubleRow)
                dst = s1T[:, m2 * N:(m2 + 1) * N]
                if m2 == 0:
                    nc.scalar.activation(dst, ps[:], AF.Relu,
                                         bias=bs1_t[m2][:] if bias_on["bs1"]
                                         else 0.0)
                else:  # relu on DVE: max(x + b, 0)
                    nc.vector.tensor_scalar(
                        out=dst, in0=ps[:],
                        scalar1=bs1_t[m2][:] if bias_on["bs1"] else 0.0,
                        scalar2=0.0, op0=OP.add, op1=OP.max)

            def sw_m(m):
                ps = ph_tile()
                for half in range(2):
                    nc.tensor.matmul(
                        ps[:, half * 512:(half + 1) * 512],
                        ws28v[:, :, m * P:(m + 1) * P],
                        s1Tv[:, :, half * 512:half * 512 + 512],
                        start=True, stop=True, perf_mode=PM.DoubleRow)
                sw = swp.tile([P, N], BF16, tag="swt")
                nc.scalar.activation(sw[:], ps[:], AF.Sigmoid,
                                     bias=bs2_t[m][:] if bias_on["bs2"]
                                     else 0.0)
                sw_m.last = sw
                nc.gpsimd.tensor_tensor(out=vvT[:, m * N:(m + 1) * N],
                                        in0=vvT[:, m * N:(m + 1) * N],
                                        in1=sw[:], op=OP.mult)

            def vch_m(m):
                # channel-gated value projection: cw is folded into wv2
                ps = ph_tile()
                for half in range(2):
                    for k in range(K4):
                        nc.tensor.matmul(
                            ps[:, half * 512:(half + 1) * 512],
                            wv2[:, k * D + m * P:k * D + (m + 1) * P],
                            vT16[:, k * N + half * 512:k * N + half * 512 + 512],
                            start=(k == 0), stop=(k == K4 - 1))
                sl = slice(m * N, (m + 1) * N)
                if bias_on["bv"]:
                    nc.vector.scalar_tensor_tensor(
                        out=vvT[:, sl], in0=ps[:], scalar=bv_t[m][:],
                        in1=vvT[:, sl], op0=OP.add, op1=OP.add)
                else:
                    nc.vector.tensor_tensor(out=vvT[:, sl], in0=ps[:],
                                            in1=vvT[:, sl], op=OP.add)

            vd_ps = [None]

            def vd_tr(t8):
                # two vd transposes share one psS slot via its bf16 view
                if t8 % 4 == 0:
                    ps = ph_tile()
                    vd_ps[0] = ps
                psb = vd_ps[0][:].bitcast(BF16)
                base = (t8 % 4) * D
                for k in range(K4):
                    nc.tensor.transpose(
                        psb[:, base + k * P:base + (k + 1) * P],
                        vvT[:, k * N + t8 * P:k * N + (t8 + 1) * P],
                        id16[:])
                if t8 % 2 == 0:
                    nc.vector.tensor_copy(vdE[:, t8 * D:(t8 + 1) * D],
                                          psb[:, base:base + D])
                else:
                    nc.scalar.copy(vdE[:, t8 * D:(t8 + 1) * D],
                                   psb[:, base:base + D])

            def proj_m(wt, bt, has_b, xt, OUT, m, move_eng):
                ps = ph_tile()
                for half in range(2):
                    for k in range(K4):
                        nc.tensor.matmul(
                            ps[:, half * 512:(half + 1) * 512],
                            wt[:, k * D + m * P:k * D + (m + 1) * P],
                            xt[:, k * N + half * 512:k * N + half * 512 + 512],
                            start=(k == 0), stop=(k == K4 - 1))
                dst = OUT[:, m * N:(m + 1) * N]
                if move_eng == "act":
                    if has_b:
                        nc.scalar.activation(dst, ps[:], AF.Identity,
                                             bias=bt[m][:])
                    else:
                        nc.scalar.copy(dst, ps[:])
                else:
                    if has_b:
                        nc.vector.tensor_scalar(out=dst, in0=ps[:],
                                                scalar1=bt[m][:], op0=OP.add)
                    else:
                        nc.vector.tensor_copy(dst, ps[:])

            for k in range(K4):
                profile_k(k)
            for m in range(K4):
                vv_m(m)
            # channel MLP, both stages in one shared psum slot
            ps_c = ph_tile()
            c1r = []
            for m2 in range(M2):
                for k in range(K4):
                    nc.tensor.matmul(
                        ps_c[:, m2:m2 + 1],
                        wc1[:, k * HID + m2 * P:k * HID + m2 * P + P],
                        profr[k][:], start=(k == 0), stop=(k == K4 - 1))
                cr = cpool.tile([P, 1], BF16, tag=f"c1r{m2}")
                nc.scalar.activation(cr[:], ps_c[:, m2:m2 + 1], AF.Relu,
                                     bias=bc1_t[m2][:] if bias_on["bc1"]
                                     else 0.0)
                c1r.append(cr)
            cw_t = []
            for m in range(K4):
                for k in range(M2):
                    nc.tensor.matmul(
                        ps_c[:, 4 + m:5 + m],
                        wc2[:, k * D + m * P:k * D + m * P + P],
                        c1r[k][:], start=(k == 0), stop=(k == M2 - 1))
                cw = cpool.tile([P, 1], F32, tag=f"cw{m}")
                nc.scalar.activation(cw[:], ps_c[:, 4 + m:5 + m], AF.Sigmoid,
                                     bias=bc2_t[m][:] if bias_on["bc2"]
                                     else 0.0)
                cw_t.append(cw)
            # fold the channel gate into the value weight (per input row)
            for k in range(K4):
                eng = nc.gpsimd if k % 2 == 0 else nc.vector
                eng.tensor_scalar_mul(out=wv2[:, k * D:(k + 1) * D],
                                      in0=wv16[:, k * D:(k + 1) * D],
                                      scalar1=cw_t[k][:])
            for m2 in range(M2):
                s1_m(m2)
            # interleave gating with the channel-path projection
            sw_m(0)
            vch_m(0)
            sw_m(1)
            vch_m(1)
            sw_m(2)
            vch_m(2)
            sw_m(3)
            vch_m(3)
            # swap the ACT tables to the exp set now (last sigmoid is done);
            # identity/relu/copy live in both sets so value-phase moves
            # after this point still need no load. Reading the last sw tile
            # pins the dummy BEHIND the final sigmoid -- without the data
            # dependency the scheduler hoists it and thrashes the tables.
            nc.scalar.activation(scr[:], sw_m.last[0:1, 0:1], AF.Exp)
            proj_m(wk16, bk_t, bias_on["bk"], kT16, kTp, 0, "dve")
            proj_m(wq16, bq_t, bias_on["bq"], qT16, qTp, 0, "act")
            for t8 in range(T8):
                vd_tr(t8)
            def proj_half(wt, bt, has_b, xt, OUT, m, half, eng):
                # half-projection through the psT bank: paced through the
                # attention stream so each position carries ~850ns of PE
                # work and the 2-deep scores ring never outruns the exps
                ps = psT[0][:]
                base = half * 512
                for k in range(K4):
                    nc.tensor.matmul(
                        ps,
                        wt[:, k * D + m * P:k * D + (m + 1) * P],
                        xt[:, k * N + base:k * N + base + 512],
                        start=(k == 0), stop=(k == K4 - 1),
                        skip_group_check=True)
                dst = OUT[:, m * N + base:m * N + base + 512]
                if eng == "act":
                    if has_b:
                        nc.scalar.activation(dst, ps, AF.Identity,
                                             bias=bt[m][:])
                    else:
                        nc.scalar.copy(dst, ps)
                else:
                    if has_b:
                        nc.vector.tensor_scalar(out=dst, in0=ps,
                                                scalar1=bt[m][:],
                                                scalar2=None, op0=OP.add)
                    else:
                        nc.vector.tensor_copy(dst, ps)

            fillers = {
                0: lambda: proj_m(wk16, bk_t, bias_on["bk"], kT16, kTp,
                                  1, "act"),
                2: lambda: proj_m(wq16, bq_t, bias_on["bq"], qT16, qTp,
                                  1, "dve"),
                4: lambda: proj_m(wk16, bk_t, bias_on["bk"], kT16, kTp,
                                  3, "act"),
            }
            for pos, (wt_, bt_, bn, xt_, OUT_, m_, eng_) in [
                (8, (wk16, bk_t, "bk", kT16, kTp, 2, "dve")),
                (14, (wk16, bk_t, "bk", kT16, kTp, 2, "dve")),
                (20, (wq16, bq_t, "bq", qT16, qTp, 2, "dve")),
                (26, (wq16, bq_t, "bq", qT16, qTp, 2, "dve")),
                (32, (wq16, bq_t, "bq", qT16, qTp, 3, "dve")),
                (38, (wq16, bq_t, "bq", qT16, qTp, 3, "dve")),
            ]:
                half_ = 0 if pos in (8, 20, 32) else 1
                fillers[pos] = (
                    lambda w=wt_, b=bt_, bn_=bn, x=xt_, O=OUT_, m=m_,
                    hf=half_, e=eng_:
                    proj_half(w, b, bias_on[bn_], x, O, m, hf, e))

            # ------- attention -------
            boB = None
            if bias_on["bo"]:
                boB = cpool.tile([P, D], F32, tag="boB")
                nc.sync.dma_start(boB[:],
                                  b_h["bo"][None, :].to_broadcast((P, D)))

            def scores_chunk(h, k8):
                p0 = (h % 2) * DH
                cc = (h // 2) * N
                ps = psS.tile([P, N], F32, tag="psS")
                for half in range(2):
                    nc.tensor.matmul(
                        ps[:, half * 512:(half + 1) * 512],
                        kTp[p0:p0 + DH, cc + k8 * P:cc + (k8 + 1) * P],
                        qTp[p0:p0 + DH,
                            cc + half * 512:cc + half * 512 + 512],
                        start=True, stop=True)
                pt = ptp.tile([P, N], BF16, tag="pt")
                if k8 in DVE_K8:
                    nc.vector.tensor_scalar(
                        out=pt[:].bitcast(I16), in0=ps[:], scalar1=SCH_K1,
                        scalar2=SCH_K2, op0=OP.mult, op1=OP.add)
                else:
                    nc.scalar.activation(pt[:], ps[:], AF.Exp, scale=0.125)
                return pt

            psD = [None]  # denominator bank, allocated at stream start
            psT = [None]  # pair-transpose staging bank

            def pv_chunk(h, k8, G, pt):
                # out[q, d] orientation: stationary = probability chunk,
                # moving = v; each ap=1 ones-matmul accumulates the softmax
                # denominator for the same q-chunk in the den bank.
                mv = vdE[:, k8 * D + h * DH:k8 * D + (h + 1) * DH]
                for q8 in range(8):
                    st = pt[:, q8 * P:(q8 + 1) * P]
                    # start=True clears the WHOLE bank's has_written bits,
                    # so only the first write of the pass may carry it; the
                    # other slots write into cleared bits (write-then-set)
                    nc.tensor.matmul(
                        G[:, q8 * DH:(q8 + 1) * DH], st, mv,
                        start=(k8 == 0 and q8 == 0), stop=(k8 == T8 - 1),
                        skip_group_check=True)
                    nc.tensor.matmul(
                        psD[0][:, h * 8 + q8:h * 8 + q8 + 1], st, ones1[:],
                        start=(k8 == 0 and q8 == 0), stop=(k8 == T8 - 1),
                        skip_group_check=True)

            rec8s = {}
            attPs = {}
            gRaws = {}

            def norm_recip(h, G):
                rec8 = rcp.tile([P, 8], F32, tag="rec8", name="rec8")
                nc.vector.reciprocal(rec8[:], psD[0][:, h * 8:(h + 1) * 8])
                rec8s[h] = rec8
                # un-normalized numerators to SBUF so GpSimd (which cannot
                # read PSUM) can do the normalize multiplies; the last head
                # is on the critical tail path, so split it across engines
                gr = rcp.tile([P, 512], BF16, tag="gRaw", name="gRaw")
                if h == H - 1:
                    nc.vector.tensor_copy(gr[:, 0:256], G[:, 0:256])
                    nc.scalar.copy(gr[:, 256:512], G[:, 256:512])
                else:
                    nc.vector.tensor_copy(gr[:], G[:])
                gRaws[h] = gr

            def norm_mult(h, G, q8s, eng):
                # normalized bf16 [token, d] staging for the pair transpose
                if h % 2 == 0 and h not in attPs:
                    attPs[h] = atp.tile([P, T8 * P], BF16, tag="attP", name="attP")
                aP = attPs[h - h % 2]
                rec8 = rec8s[h]
                gr = gRaws[h]
                for q8 in q8s:
                    dst = aP[:, q8 * P + (h % 2) * DH:
                             q8 * P + (h % 2) * DH + DH]
                    nc.gpsimd.tensor_scalar_mul(
                        out=dst, in0=gr[:, q8 * DH:(q8 + 1) * DH],
                        scalar1=rec8[:, q8:q8 + 1])

            def pair_transpose(j):
                aP = attPs[2 * j]
                psTb = psT[0][:].bitcast(BF16)
                for q8 in range(8):
                    nc.tensor.transpose(psTb[:, q8 * P:(q8 + 1) * P],
                                        aP[:, q8 * P:(q8 + 1) * P], id16[:])

            def pair_copy(j, eng):
                psTb = psT[0][:].bitcast(BF16)
                if eng == "act":
                    nc.scalar.copy(outT[:, j * N:(j + 1) * N], psTb[:, 0:N])
                else:
                    nc.vector.tensor_copy(outT[:, j * N:(j + 1) * N],
                                          psTb[:, 0:N])

            acc = big.tile([P, T8 * D], BF16, tag="acc")

            def partial012(q8):
                # out[q8] partials: k=0,1,2 chunks of outT.T @ Wo. While
                # scores still run, use the psT bank so they keep both psS
                # buffers; the last two partials run after the final scores
                # chunk and take a psS tile (two groups in one tile),
                # breaking the psT write->copy->write serialization.
                if q8 < 6:
                    ps = psT[0][:]
                elif q8 % 2 == 0:
                    partial012.ps6 = psS.tile([P, N], F32, tag="psS")
                    ps = partial012.ps6[:, 0:D]
                else:
                    ps = partial012.ps6[:, D:2 * D]
                for k in range(3):
                    nc.tensor.matmul(
                        ps,
                        outT[:, k * N + q8 * P:k * N + (q8 + 1) * P],
                        wo16[:, k * D:(k + 1) * D],
                        start=(k == 0), stop=(k == 2),
                        skip_group_check=True)
                at = acc[:, q8 * D:(q8 + 1) * D]
                if bias_on["bo"]:
                    nc.vector.tensor_tensor(out=at, in0=ps, in1=boB[:],
                                            op=OP.add)
                else:
                    nc.vector.tensor_copy(at, ps)

            # flat software pipeline across all heads: a continuous stream
            # of scores->exp chunks with PV trailing PV_LAG chunks behind.
            TOT = H * T8
            pts = [None] * TOT
            Gs = [None] * H
            # first position strictly after pair 2's outT copy (head 5
            # done at 8*5+7+PV_LAG, copy 5 positions later)
            p012_base = 8 * 5 + 7 + PV_LAG + 6
            for q8 in range(8):
                fillers[p012_base + q8] = (lambda qq=q8: partial012(qq))
            recip_at = {}
            mult_at = {}
            transp_at = {}
            copy_at = {}
            psG = None
            for i in range(TOT + PV_LAG + 10):
                # norm/transpose/copy events first: several share the psT
                # bank with the half-projection fillers, and emission order
                # is what Tile's dependency tracking sees
                if i in recip_at:
                    norm_recip(recip_at[i], Gs[recip_at[i]])
                for key in (i, i + 0.5, i + 0.75):
                    if key in mult_at:
                        hh, q8s, eng = mult_at[key]
                        norm_mult(hh, Gs[hh], q8s, eng)
                if i in transp_at:
                    pair_transpose(transp_at[i])
                if i in copy_at:
                    pair_copy(copy_at[i], "dve")
                if i in fillers:
                    fillers[i]()
                j = i - PV_LAG
                if j == 0:
                    # all psB users (value-path/projection fillers) are done;
                    # hand its banks to the attention pools
                    psB.release()
                    _pools[:] = [psS]
                    _pi[0] = 0
                    psG = tc.alloc_tile_pool(name="psG", bufs=2,
                                             space="PSUM", side="left")
                    psDp = tc.alloc_tile_pool(name="psD", bufs=1,
                                              space="PSUM", side="left")
                    psTp = tc.alloc_tile_pool(name="psT", bufs=1,
                                              space="PSUM", side="left")
                    psD[0] = psDp.tile([P, 512], F32, tag="psD", name="psDt")
                    psT[0] = psTp.tile([P, 512], F32, tag="psT", name="psTt")
                if 0 <= j < TOT:
                    h, k8 = divmod(j, T8)
                    if k8 == 0:
                        Gs[h] = psG.tile([P, 512], F32, tag="G", name="Gt")
                    # PV emitted before this position's scores: its pt is
                    # long ready, and the scores' wait on the psS ring then
                    # lands ~220ns later, giving the exps more deadline
                    pv_chunk(h, k8, Gs[h], pts[j])
                    pts[j] = None
                    if k8 == T8 - 1 and h < H - 1:
                        recip_at[i + 1] = h
                        mult_at[i + 2] = (h, range(0, 4), "pool")
                        mult_at[i + 3] = (h, range(4, 8), "pool")
                        if h % 2 == 1:
                            transp_at[i + 4] = h // 2
                            copy_at[i + 5] = h // 2
                if i < TOT:
                    h, k8 = divmod(i, T8)
                    pts[i] = scores_chunk(h, k8)

            # ------- tail: last head's norm, pair 3, k=3 partials, out -----
            h7 = H - 1
            norm_recip(h7, Gs[h7])
            psTb = psT[0][:].bitcast(BF16)
            aP3 = attPs[6]
            # interleave: 4 mults -> 4 transposes -> first outT half-copy,
            # so the first k3 matmuls start while the second half is still
            # normalizing
            norm_mult(h7, Gs[h7], range(0, 4), "pool")
            for q8 in range(4):
                nc.tensor.transpose(psTb[:, q8 * P:(q8 + 1) * P],
                                    aP3[:, q8 * P:(q8 + 1) * P], id16[:])
            nc.vector.tensor_copy(outT[:, 3 * N:3 * N + 512], psTb[:, 0:512])
            norm_mult(h7, Gs[h7], range(4, 8), "pool")
            for q8 in range(4, 8):
                nc.tensor.transpose(psTb[:, q8 * P:(q8 + 1) * P],
                                    aP3[:, q8 * P:(q8 + 1) * P], id16[:])
            nc.scalar.copy(outT[:, 3 * N + 512:4 * N], psTb[:, 512:N])

            def tail_q8(q8):
                # per-q8 output group: k3 matmul + identity-matmul fold of
                # the k=0..2 partial (acc) on the idle PE, then a plain
                # copy (ACT/DVE alternating) and its own DMA (SP/Pool
                # alternating). Rotates psG halves + psS halves: a 6-slot
                # ring, so no group ever waits on a copy.
                if q8 % 4 < 2:
                    g = psG.tile([P, 512], F32, tag="G", name="Gt")
                    ps = g[:]
                else:
                    if q8 % 4 == 2:
                        tail_q8.ps = psS.tile([P, N], F32, tag="psS")
                    ps = tail_q8.ps[:, (q8 % 4 - 2) * D:(q8 % 4 - 1) * D]
                nc.tensor.matmul(
                    ps,
                    outT[:, 3 * N + q8 * P:3 * N + (q8 + 1) * P],
                    wo16[:, 3 * D:4 * D],
                    start=True, stop=False, skip_group_check=True)
                nc.tensor.matmul(
                    ps, id16[:], acc[:, q8 * D:(q8 + 1) * D],
                    start=False, stop=True, skip_group_check=True)
                sl = slice(q8 * D, (q8 + 1) * D)
                if q8 % 2 == 0:
                    nc.vector.tensor_copy(stg[:, sl], ps)
                    dma = nc.scalar
                else:
                    nc.scalar.copy(stg[:, sl], ps)
                    dma = nc.sync
                dma.dma_start(out_h[q8 * P:(q8 + 1) * P, :], stg[:, sl])

            for q8 in range(8):
                tail_q8(q8)
            if DEBUG:
                for nm, t in [("vvT", vvT), ("qTp", qTp), ("kTp", kTp),
                              ("vdE", vdE), ("outT", outT)]:
                    nc.sync.dma_start(dbg_h[nm][:], t[:])
            psS.release()
            psTp.release()
            psDp.release()
            psG.release()

    nc.finalize()
    return nc


def get_nc(bias_key=frozenset()):
    if bias_key not in _CACHE:
        bias_on = {nm: (nm in bias_key) for nm in BIAS_NAMES}
        _CACHE[bias_key] = _build(bias_on)
    return _CACHE[bias_key]


def make_in_maps(inputs):
    """Host-side marshalling: transpose + quantize, shard over batch."""
    if "key_in" not in inputs and "key" in inputs:
        inputs = dict(inputs)
        inputs["key_in"] = inputs.pop("key")
    f32 = np.float32
    bs1_eff = (np.asarray(inputs["bs1"], np.float64)
               + np.asarray(inputs["bv"], np.float64)
               @ np.asarray(inputs["Ws1"], np.float64)).astype(f32)
    inputs = dict(inputs)
    inputs["bs1"] = bs1_eff
    bias_key = frozenset(
        nm for nm in BIAS_NAMES
        if np.any(np.asarray(inputs[nm], f32) != 0.0))
    shared = {
        "Wv16": np.asarray(inputs["Wv"], f32).astype(BF16NP),
        "Wq16": np.asarray(inputs["Wq"], f32).astype(BF16NP),
        "Wk16": np.asarray(inputs["Wk"], f32).astype(BF16NP),
        "Wo16": np.asarray(inputs["Wo"], f32).astype(BF16NP),
        "Wvs18": (np.asarray(inputs["Wv"], np.float64)
                  @ np.asarray(inputs["Ws1"], np.float64)).astype(F8NP),
        "Ws28": np.asarray(inputs["Ws2"], f32).astype(F8NP),
        "Wc1": np.asarray(inputs["Wc1"], f32).astype(BF16NP),
        "Wc2": np.asarray(inputs["Wc2"], f32).astype(BF16NP),
    }
    for nm in bias_key:
        shared[nm] = np.ascontiguousarray(np.asarray(inputs[nm], f32))
    q = np.asarray(inputs["query"], f32)
    k = np.asarray(inputs["key_in"], f32)
    v = np.asarray(inputs["value"], f32)
    in_maps = []
    for c in range(NCORES):
        m = dict(shared)
        m["vT16"] = np.ascontiguousarray(v[c].T).astype(BF16NP)
        m["vT8"] = np.ascontiguousarray(v[c].T).astype(F8NP)
        m["qT16"] = np.ascontiguousarray(q[c].T).astype(BF16NP)
        m["kT16"] = np.ascontiguousarray(k[c].T).astype(BF16NP)
        in_maps.append(m)
    return in_maps, bias_key


def kernel(**inputs):
    in_maps, bias_key = make_in_maps(inputs)
    nc = get_nc(bias_key)
    res = run_bass_kernel_spmd(nc, in_maps, core_ids=list(range(NCORES)))
    return np.stack(
        [res.results[c]["outB"].astype(np.float32) for c in range(NCORES)],
        axis=0)
